# revision 1
# baseline (speedup 1.0000x reference)
"""Trainium2 Bass kernel for sparse-graph GCN (nn_HGC_LRN_25237227832003).

Pipeline per the reference:
  phi = MLP(col indices)                      [S=4096, D=256]  (host, tiny)
  h0  = (mask @ phi) / cnt                    [N=20000, D]     (device matmul)
  3x: h = relu(segment_sum(h[row]*attr, col) @ W)              (device)

Strategy: the edge aggregation is multiplication by a sparse matrix
A[src, dst] = sum(attr over duplicate edges).  Build A densely on the host
(bf16, per-core slice of destinations) and run the aggregation as a pure
streaming GEMM on the Tensor engine -- no gathers.  The aggregation matmul
is arranged to produce the *transposed* aggregate aggT[feat, dst], which
feeds the W matmul directly as lhsT, so no on-chip transposes anywhere.

Sharding: destinations (rows) split across 8 cores, 2500 each, padded to
2560.  h is replicated each layer via AllGather (bf16, Shared-output fast
path) in a [128, 20*256] partition-major layout so each core reloads it
with 8 large DMAs.
"""
import sys
import numpy as np

for _p in ("/opt/trn_rl_repo",):
    if _p not in sys.path:
        sys.path.append(_p)

import ml_dtypes

N, S, E, D = 20000, 4096, 640000, 256
NCORES = 8
RPC = 2500            # real rows per core
NBLK = 20             # 128-row dest blocks per core
RPC_PAD = NBLK * 128  # 2560
NPAD = RPC_PAD * NCORES  # 20480
KCH = S // 128        # 32 stage-1 contraction chunks
NLAYERS = 3
NCH = NPAD // 128     # 160 source chunks of 128
NDT = RPC_PAD // 512  # 5 dest tiles of 512 per core
NG = 8                # A chunk groups per dest tile
GJ = NCH // NG        # 20 source chunks per group

FP8A = True           # A matrix in fp8 e4m3 (halves A DMA; rel err ~1.1e-2)
FP8MASK = True        # 0/1 mask in fp8 + exact per-row 1/cnt scale (f32)

_nc_cache = {}


def _build_nc(reps=1):
    import concourse.bass as bass
    import concourse.bacc as bacc
    import concourse.tile as tile
    import concourse.mybir as mybir

    bf16 = mybir.dt.bfloat16
    f32 = mybir.dt.float32
    adt = mybir.dt.float8e4 if FP8A else bf16
    mdt = mybir.dt.float8e4 if FP8MASK else bf16

    nc = bacc.Bacc("TRN2", target_bir_lowering=False, debug=False,
                   num_devices=NCORES)

    mask_d = nc.dram_tensor("mask_blk", [NBLK, 128, S], mdt, kind="ExternalInput")
    phi_d = nc.dram_tensor("phi_sb", [128, KCH * D], bf16, kind="ExternalInput")
    w_d = nc.dram_tensor("w_sb", [128, 2 * D], bf16, kind="ExternalInput")
    a_d = nc.dram_tensor("a_til", [NDT * NG, 128, GJ * 512], adt,
                         kind="ExternalInput")
    cinv_d = nc.dram_tensor("cinv", [128, NBLK], f32, kind="ExternalInput")
    out_d = nc.dram_tensor("out_ext", [RPC_PAD, D], f32, kind="ExternalOutput")

    rg = [list(range(NCORES))]

    with tile.TileContext(nc) as tc:
        with tc.tile_pool(name="const", bufs=1) as cst, \
             tc.tile_pool(name="maskp", bufs=3) as maskp, \
             tc.tile_pool(name="apool", bufs=3) as apool, \
             tc.tile_pool(name="hsp", bufs=1) as hsp, \
             tc.tile_pool(name="aggp", bufs=3) as aggp, \
             tc.tile_pool(name="hop", bufs=4) as hop, \
             tc.tile_pool(name="psA", bufs=2, space="PSUM") as psA, \
             tc.tile_pool(name="psW", bufs=2, space="PSUM") as psW, \
             tc.tile_pool(name="dram", bufs=1, space="DRAM") as dram:

            phi_t = cst.tile([128, KCH * D], bf16, tag="phi")
            nc.sync.dma_start(out=phi_t[:], in_=phi_d.ap())
            w_t = cst.tile([128, 2 * D], bf16, tag="w")
            nc.sync.dma_start(out=w_t[:], in_=w_d.ap())
            cinv_t = cst.tile([128, NBLK], f32, tag="cinv")
            nc.sync.dma_start(out=cinv_t[:], in_=cinv_d.ap())

            # ---- stage 1: h0 = scaled_mask.T @ phi ----
            for rep in range(reps):
              # gathered h, partition-major: ag_in[p, nb*D+f] = h[nb*128+p, f]
              ag_in = [dram.tile([128, NBLK * D], bf16, name=f"ag_in{rep}_{l}")
                       for l in range(NLAYERS)]
              ag_out = [dram.tile([NCORES, 128, NBLK * D], bf16,
                                  name=f"ag_out{rep}_{l}", addr_space="Shared")
                        for l in range(NLAYERS)]
              for nb in range(NBLK):
                mt = maskp.tile([128, S], mdt, tag="mt")
                nc.sync.dma_start(out=mt[:], in_=mask_d.ap()[nb])
                acc = psW.tile([128, D], f32, space="PSUM", tag="pw")
                for sc in range(KCH):
                    nc.tensor.matmul(
                        out=acc[:],
                        lhsT=mt[:, sc * 128:(sc + 1) * 128],
                        rhs=phi_t[:, sc * D:(sc + 1) * D],
                        start=(sc == 0), stop=(sc == KCH - 1),
                    )
                h0 = hop.tile([128, D], bf16, tag="hn")
                nc.vector.tensor_scalar(
                    out=h0[:], in0=acc[:], scalar1=cinv_t[:, nb:nb + 1],
                    scalar2=None, op0=mybir.AluOpType.mult,
                )
                nc.sync.dma_start(out=ag_in[0][:, nb * D:(nb + 1) * D], in_=h0[:])

              # ---- layers ----
              for l in range(NLAYERS):
                nc.gpsimd.collective_compute(
                    "AllGather", mybir.AluOpType.bypass, replica_groups=rg,
                    ins=[ag_in[l].opt()], outs=[ag_out[l].opt()],
                )
                # replicated h in SBUF: hs[c0][p, j*D+f] = h[(c0*GJ+j)*128+p, f]
                hs = []
                for c0 in range(NCORES):
                    ht = hsp.tile([128, NBLK * D], bf16, tag=f"hs{c0}",
                                  name=f"hs{c0}")
                    nc.sync.dma_start(out=ht[:], in_=ag_out[l][c0])
                    hs.append(ht)

                for dt in range(NDT):
                    pa = [psA.tile([128, 512], f32, space="PSUM",
                                   tag=f"pa{fc}", name=f"pa{fc}")
                          for fc in range(2)]
                    for g in range(NG):
                        at = apool.tile([128, GJ * 512], adt, tag="at")
                        nc.sync.dma_start(out=at[:], in_=a_d.ap()[dt * NG + g])
                        for j in range(GJ):
                            c = g * GJ + j
                            hsrc = hs[c // NBLK]
                            coff = (c % NBLK) * D
                            for fc in range(2):
                                nc.tensor.matmul(
                                    out=pa[fc][:],
                                    lhsT=hsrc[:, coff + fc * 128:
                                              coff + (fc + 1) * 128],
                                    rhs=at[:, j * 512:(j + 1) * 512],
                                    start=(c == 0), stop=(c == NCH - 1),
                                )
                    aggT = aggp.tile([128, 1024], bf16, tag="aggT")
                    nc.vector.tensor_copy(out=aggT[:, 0:512], in_=pa[0][:])
                    nc.vector.tensor_copy(out=aggT[:, 512:1024], in_=pa[1][:])
                    for sb in range(4):
                        pw = psW.tile([128, D], f32, space="PSUM", tag="pw")
                        for fc in range(2):
                            nc.tensor.matmul(
                                out=pw[:],
                                lhsT=aggT[:, fc * 512 + sb * 128:
                                          fc * 512 + (sb + 1) * 128],
                                rhs=w_t[:, fc * D:(fc + 1) * D],
                                start=(fc == 0), stop=(fc == 1),
                            )
                        nb = dt * 4 + sb
                        if l < NLAYERS - 1:
                            hn = hop.tile([128, D], bf16, tag="hn")
                            nc.vector.tensor_scalar(
                                out=hn[:], in0=pw[:], scalar1=0.0, scalar2=None,
                                op0=mybir.AluOpType.max,
                            )
                            nc.sync.dma_start(
                                out=ag_in[l + 1][:, nb * D:(nb + 1) * D],
                                in_=hn[:])
                        else:
                            ho = hop.tile([128, D], f32, tag="ho")
                            nc.vector.tensor_scalar(
                                out=ho[:], in0=pw[:], scalar1=0.0, scalar2=None,
                                op0=mybir.AluOpType.max,
                            )
                            nc.sync.dma_start(
                                out=out_d.ap()[nb * 128:(nb + 1) * 128, :],
                                in_=ho[:])

    nc.compile()
    return nc


def _prep_inputs(init, edge_index, edge_attr, w1, b1, w2, b2, W):
    bf = ml_dtypes.bfloat16
    adt = ml_dtypes.float8_e4m3 if FP8A else bf
    mdt = ml_dtypes.float8_e4m3 if FP8MASK else bf

    # phi = MLP(column indices), tiny -- fp32 on host
    idx = np.arange(S, dtype=np.float32)[:, None]
    phi = np.maximum(idx @ np.asarray(w1, np.float32) + np.asarray(b1, np.float32),
                     0.0) @ np.asarray(w2, np.float32) + np.asarray(b2, np.float32)
    phi_sb = np.ascontiguousarray(
        phi.reshape(KCH, 128, D).transpose(1, 0, 2).reshape(128, KCH * D)
    ).astype(bf)

    Wf = np.asarray(W, np.float32)
    w_sb = np.ascontiguousarray(
        Wf.reshape(2, 128, D).transpose(1, 0, 2).reshape(128, 2 * D)).astype(bf)

    # ---- dense A[src_pad, dst_local] per destination core, tiled ----
    row = np.asarray(edge_index[0], np.int64)
    col = np.asarray(edge_index[1], np.int64)
    attr = np.asarray(edge_attr, np.float32)
    src_pad = (row // RPC) * RPC_PAD + (row % RPC)
    core = col // RPC
    dl = col % RPC
    flat = src_pad * RPC_PAD + dl

    a_tiles = []
    for c in range(NCORES):
        m = core == c
        Ac = np.zeros(NPAD * RPC_PAD, np.float32)
        np.add.at(Ac, flat[m], attr[m])
        At = (Ac.reshape(NG, GJ, 128, NDT, 512)
              .transpose(3, 0, 2, 1, 4)
              .reshape(NDT * NG, 128, GJ * 512))
        a_tiles.append(np.ascontiguousarray(At).astype(adt))

    # ---- 0/1 mask blocks (fp8-exact) + 1/cnt row scales per core ----
    init = np.asarray(init)
    in_maps = []
    for c in range(NCORES):
        rows_c = init[c * RPC:(c + 1) * RPC]
        m = (rows_c != 0)
        cnt = m.sum(axis=1)
        sp = np.zeros((RPC_PAD, S), np.float32)
        if FP8MASK:
            sp[:RPC] = m
        else:
            sp[:RPC] = m / np.maximum(cnt, 1.0)[:, None]
        mask_blk = np.ascontiguousarray(
            sp.reshape(NBLK, 128, KCH, 128).transpose(0, 3, 2, 1)
            .reshape(NBLK, 128, S)).astype(mdt)
        cinv = np.ones(RPC_PAD, np.float32)
        cinv[:RPC] = 1.0 / np.maximum(cnt, 1.0)
        cinv_sb = np.ascontiguousarray(cinv.reshape(NBLK, 128).T)
        in_maps.append({
            "mask_blk": mask_blk,
            "phi_sb": phi_sb,
            "w_sb": w_sb,
            "a_til": a_tiles[c],
            "cinv": cinv_sb if FP8MASK else np.ones((128, NBLK), np.float32),
        })
    return in_maps


def kernel(init, edge_index, edge_attr, w1, b1, w2, b2, W, _trace=False):
    from concourse.bass_utils import run_bass_kernel_spmd

    in_maps = _prep_inputs(init, edge_index, edge_attr, w1, b1, w2, b2, W)
    if "nc" not in _nc_cache:
        _nc_cache["nc"] = _build_nc()
    nc = _nc_cache["nc"]
    res = run_bass_kernel_spmd(nc, in_maps, core_ids=list(range(NCORES)),
                               trace=_trace)
    kernel.last_results = res
    kernel.last_in_maps = in_maps
    kernel.last_nc = nc
    full = np.empty((N, D), np.float32)
    for c in range(NCORES):
        full[c * RPC:(c + 1) * RPC] = res.results[c]["out_ext"][:RPC]
    return full

